# revision 10
# baseline (speedup 1.0000x reference)
"""Expert-parallel MoE MLP kernel for TRN2 (8 NeuronCores).

Reference computation (all experts, dense routing):
    hidden = einsum("bnd,edh->benh", x, w1); hidden = gelu(hidden)
    out    = einsum("benh,ehd->bnde", hidden, w2)        # [b, n, d4, e]

Sharding: expert-parallel, 2 experts per core (16 experts / 8 cores); x is
replicated. Each core computes, for its experts e:
    hT[e] = gelu(W1[e].T @ X.T)        # [h, tok] layout, h on partitions
    outT[e] = W2[e].T @ hT[e]          # [d4, tok] layout
which keeps the contraction dim on SBUF partitions for both matmuls with no
on-device transposes.

The whole data path is bf16 (PSUM accumulation stays f32): bf16 matmuls run
at the same 1 row/cycle as fp32r but allow a 1024-wide moving operand (halved
instruction count and per-instruction overhead), enable fast weight load, and
halve all DMA traffic including the output (upcast to f32 on the host;
end-to-end quantization error ~4e-3, well under the 2e-2 gate). DMA descriptors
are consolidated into few dma_starts (each costs ~600ns of serialized
sequencer config time) with the first token tile's data queued ahead of
everything else. The [e, d4, tok] device layout is re-interleaved to
[b, n, d4, e] on the host.
"""

import sys

import numpy as np

for _p in ("/opt/trn_rl_repo", "/root/.axon_site/_ro/trn_rl_repo"):
    if _p not in sys.path:
        sys.path.append(_p)

import ml_dtypes

import concourse.bacc as bacc
import concourse.mybir as mybir
import concourse.tile as tile
from concourse.bass_utils import run_bass_kernel_spmd

F32 = mybir.dt.float32
BF16 = mybir.dt.bfloat16
NP_BF16 = ml_dtypes.bfloat16

N_CORES = 8
E = 16                 # total experts
E_LOC = E // N_CORES   # experts per core
D = 512                # model dim (contraction of mm1)
H = 512                # hidden dim (contraction of mm2)
D4 = 128               # output dim per expert
NTOK = 4 * 2048        # tokens
TT = 512               # token tile (matmul moving free dim)
P = 128


def _build_program():
    nc = bacc.Bacc("TRN2", target_bir_lowering=False, debug=False)
    xT = nc.declare_dram_parameter("xT", [D, NTOK], BF16, isOutput=False)
    w1 = nc.declare_dram_parameter("w1", [E_LOC, D, H], BF16, isOutput=False)
    w2 = nc.declare_dram_parameter("w2", [E_LOC, H, D4], BF16, isOutput=False)
    outT = nc.declare_dram_parameter("outT", [E_LOC, D4, NTOK], BF16, isOutput=True)

    gelu = mybir.ActivationFunctionType.Gelu
    n_dt = D // P   # 4 k-tiles of mm1
    n_ht = H // P   # 4 k-tiles of mm2

    with tile.TileContext(nc) as tc:
        with (
            tc.tile_pool(name="wpool", bufs=1) as wpool,
            tc.tile_pool(name="xpool", bufs=4) as xpool,
            tc.tile_pool(name="hpool", bufs=2) as hpool,
            tc.tile_pool(name="opool", bufs=4) as opool,
            tc.tile_pool(name="ps1p", bufs=4, space="PSUM") as ps1p,
            tc.tile_pool(name="ps2p", bufs=3, space="PSUM") as ps2p,
        ):
            # Weights resident in SBUF for the whole kernel, natural layout.
            w1_sb = wpool.tile([P, E_LOC, n_dt, H], BF16, name="w1_sb", tag="w1")
            w1_r = w1.rearrange("e (dt p) h -> p e dt h", p=P)
            w2_sb = wpool.tile([P, E_LOC, n_ht, D4], BF16, name="w2_sb", tag="w2")
            w2_r = w2.rearrange("e (ht p) d -> p e ht d", p=P)
            xT_r = xT.rearrange("(dt p) n -> p dt n", p=P)

            x_tiles = {}

            def load_x(t):
                tok = slice(t * TT, (t + 1) * TT)
                x_sb = xpool.tile([P, n_dt, TT], BF16, name="x_sb", tag="x")
                nc.sync.dma_start(x_sb, xT_r[:, :, tok])
                x_tiles[t] = x_sb

            # Startup: the first matmul needs only x0[dt0] + w1[e0][dt0];
            # those two DMAs go first (each dma_start costs ~640ns of serial
            # sequencer config, so the critical ones must lead the queue).
            tok0 = slice(0, TT)
            x0_sb = xpool.tile([P, n_dt, TT], BF16, name="x_sb", tag="x")
            nc.sync.dma_start(x0_sb[:, 0], xT_r[:, 0, tok0])
            nc.sync.dma_start(w1_sb[:, 0, 0], w1_r[:, 0, 0])
            nc.sync.dma_start(x0_sb[:, 1], xT_r[:, 1, tok0])
            nc.sync.dma_start(w1_sb[:, 0, 1], w1_r[:, 0, 1])
            nc.sync.dma_start(x0_sb[:, 2:4], xT_r[:, 2:4, tok0])
            nc.sync.dma_start(w1_sb[:, 0, 2:4], w1_r[:, 0, 2:4])
            x_tiles[0] = x0_sb
            nc.sync.dma_start(w2_sb[:, 0], w2_r[:, 0])
            for e in range(1, E_LOC):
                nc.sync.dma_start(w1_sb[:, e], w1_r[:, e])
                nc.sync.dma_start(w2_sb[:, e], w2_r[:, e])

            last_t = NTOK // TT - 1
            for t in range(NTOK // TT):
                tok = slice(t * TT, (t + 1) * TT)
                if t not in x_tiles:
                    load_x(t)
                x_sb = x_tiles.pop(t)
                hT_tiles = []
                for e in range(E_LOC):
                    hT_sb = hpool.tile([P, n_ht, TT], BF16, name="hT_sb", tag="h")
                    for ht in range(n_ht):
                        ps1 = ps1p.tile([P, TT], F32, name="ps1", tag="ps1")
                        for dt_i in range(n_dt):
                            nc.tensor.matmul(
                                ps1,
                                w1_sb[:, e, dt_i, ht * P : (ht + 1) * P],
                                x_sb[:, dt_i],
                                start=(dt_i == 0),
                                stop=(dt_i == n_dt - 1),
                            )
                        nc.scalar.activation(hT_sb[:, ht, :], ps1, gelu)
                    hT_tiles.append(hT_sb)
                for e in range(E_LOC):
                    ps2 = ps2p.tile([P, TT], F32, name="ps2", tag="ps2")
                    for ht in range(n_ht):
                        nc.tensor.matmul(
                            ps2,
                            w2_sb[:, e, ht, :],
                            hT_tiles[e][:, ht, :],
                            start=(ht == 0),
                            stop=(ht == n_ht - 1),
                        )
                    o_sb = opool.tile([P, TT], BF16, name="o_sb", tag="o")
                    if t == last_t and e == E_LOC - 1:
                        # Final drain: quarter the last output across DVE and
                        # Scalar so the copies and DMAs pipeline in parallel.
                        q = TT // 4
                        for qi, eng in ((0, nc.vector.tensor_copy),
                                        (1, nc.scalar.copy),
                                        (2, nc.vector.tensor_copy),
                                        (3, nc.scalar.copy)):
                            hv = slice(qi * q, (qi + 1) * q)
                            eng(o_sb[:, hv], ps2[:, hv])
                            nc.sync.dma_start(
                                outT[e, :, t * TT + hv.start : t * TT + hv.stop],
                                o_sb[:, hv],
                            )
                    else:
                        nc.vector.tensor_copy(o_sb, ps2)
                        nc.sync.dma_start(outT[e, :, tok], o_sb)

    nc.finalize()
    return nc


_NC = None


def _get_program():
    global _NC
    if _NC is None:
        _NC = _build_program()
    return _NC


def _prep_in_maps(x, w1, w2):
    """Host-side bf16 cast + transpose; returns per-core input maps."""
    X = np.ascontiguousarray(x.reshape(NTOK, D)).astype(np.float32, copy=False)
    xT = np.ascontiguousarray(X.T.astype(NP_BF16))

    in_maps = []
    for c in range(N_CORES):
        w1c = np.ascontiguousarray(
            w1[c * E_LOC : (c + 1) * E_LOC].astype(NP_BF16)
        )
        w2c = np.ascontiguousarray(
            w2[c * E_LOC : (c + 1) * E_LOC].astype(NP_BF16)
        )
        in_maps.append({"xT": xT, "w1": w1c, "w2": w2c})
    return in_maps


def kernel(x: np.ndarray, w1: np.ndarray, w2: np.ndarray, **_) -> np.ndarray:
    """Full inputs in, full output out; expert-parallel across 8 NeuronCores."""
    nc = _get_program()
    in_maps = _prep_in_maps(x, w1, w2)
    res = run_bass_kernel_spmd(nc, in_maps, list(range(N_CORES)))

    full = np.stack(
        [res.results[c]["outT"].astype(np.float32) for c in range(N_CORES)], axis=0
    )
    full = full.reshape(E, D4, NTOK)              # [e, d4, tok]
    out = full.transpose(2, 1, 0)                 # [tok, d4, e]
    return np.ascontiguousarray(out.reshape(4, 2048, D4, E), dtype=np.float32)


# revision 12
# speedup vs baseline: 1.0066x; 1.0066x over previous
"""Expert-parallel MoE MLP kernel for TRN2 (8 NeuronCores).

Reference computation (all experts, dense routing):
    hidden = einsum("bnd,edh->benh", x, w1); hidden = gelu(hidden)
    out    = einsum("benh,ehd->bnde", hidden, w2)        # [b, n, d4, e]

Sharding: expert-parallel, 2 experts per core (16 experts / 8 cores); x is
replicated. Each core computes, for its experts e:
    hT[e] = gelu(W1[e].T @ X.T)        # [h, tok] layout, h on partitions
    outT[e] = W2[e].T @ hT[e]          # [d4, tok] layout
which keeps the contraction dim on SBUF partitions for both matmuls with no
on-device transposes.

The whole data path is bf16 (PSUM accumulation stays f32): bf16 matmuls run
at the same 1 row/cycle as fp32r (216ns vs 227ns per 512-row matmul measured
- fast weight load hides the weight-load bubble fp32r pays), and bf16 halves
all DMA traffic including the output (upcast to f32 on the host; end-to-end
quantization error ~4e-3, well under the 2e-2 gate). fp8 DoubleRow was
measured at only ~2x fp32r MAC rate on this hardware (not the 4x the cost
model claims), which makes the 3-term hi/lo error-compensation scheme the
2e-2 gate requires a net loss - so bf16 it is. DMA is consolidated into few
dma_starts (each costs ~640ns of serialized sequencer config time) with the
first token tile's critical slices queued ahead of everything else. The
[e, d4, tok] device layout is re-interleaved to [b, n, d4, e] on the host.
"""

import sys

import numpy as np

for _p in ("/opt/trn_rl_repo", "/root/.axon_site/_ro/trn_rl_repo"):
    if _p not in sys.path:
        sys.path.append(_p)

import ml_dtypes

import concourse.bacc as bacc
import concourse.mybir as mybir
import concourse.tile as tile
from concourse.bass_utils import run_bass_kernel_spmd

F32 = mybir.dt.float32
BF16 = mybir.dt.bfloat16
NP_BF16 = ml_dtypes.bfloat16

N_CORES = 8
E = 16                 # total experts
E_LOC = E // N_CORES   # experts per core
D = 512                # model dim (contraction of mm1)
H = 512                # hidden dim (contraction of mm2)
D4 = 128               # output dim per expert
NTOK = 4 * 2048        # tokens
TT = 512               # token tile (matmul moving free dim)
P = 128


def _build_program():
    nc = bacc.Bacc("TRN2", target_bir_lowering=False, debug=False)
    xT = nc.declare_dram_parameter("xT", [D, NTOK], BF16, isOutput=False)
    w1 = nc.declare_dram_parameter("w1", [E_LOC, D, H], BF16, isOutput=False)
    w2 = nc.declare_dram_parameter("w2", [E_LOC, H, D4], BF16, isOutput=False)
    outT = nc.declare_dram_parameter("outT", [E_LOC, D4, NTOK], BF16, isOutput=True)

    gelu = mybir.ActivationFunctionType.Gelu
    n_dt = D // P   # 4 k-tiles of mm1
    n_ht = H // P   # 4 k-tiles of mm2

    with tile.TileContext(nc) as tc:
        with (
            tc.tile_pool(name="wpool", bufs=1) as wpool,
            tc.tile_pool(name="xpool", bufs=4) as xpool,
            tc.tile_pool(name="hpool", bufs=2) as hpool,
            tc.tile_pool(name="opool", bufs=4) as opool,
            tc.tile_pool(name="ps1p", bufs=4, space="PSUM") as ps1p,
            tc.tile_pool(name="ps2p", bufs=3, space="PSUM") as ps2p,
        ):
            # Weights resident in SBUF for the whole kernel, natural layout.
            w1_sb = wpool.tile([P, E_LOC, n_dt, H], BF16, name="w1_sb", tag="w1")
            w1_r = w1.rearrange("e (dt p) h -> p e dt h", p=P)
            w2_sb = wpool.tile([P, E_LOC, n_ht, D4], BF16, name="w2_sb", tag="w2")
            w2_r = w2.rearrange("e (ht p) d -> p e ht d", p=P)
            xT_r = xT.rearrange("(dt p) n -> p dt n", p=P)

            x_tiles = {}

            def load_x(t):
                tok = slice(t * TT, (t + 1) * TT)
                x_sb = xpool.tile([P, n_dt, TT], BF16, name="x_sb", tag="x")
                nc.sync.dma_start(x_sb, xT_r[:, :, tok])
                x_tiles[t] = x_sb

            # Startup: the first matmul needs only x0[dt0] + w1[e0][dt0];
            # those two DMAs go first (each dma_start costs ~640ns of serial
            # sequencer config, so the critical ones must lead the queue).
            tok0 = slice(0, TT)
            x0_sb = xpool.tile([P, n_dt, TT], BF16, name="x_sb", tag="x")
            nc.sync.dma_start(x0_sb[:, 0], xT_r[:, 0, tok0])
            nc.sync.dma_start(w1_sb[:, 0, 0], w1_r[:, 0, 0])
            nc.sync.dma_start(x0_sb[:, 1], xT_r[:, 1, tok0])
            nc.sync.dma_start(w1_sb[:, 0, 1], w1_r[:, 0, 1])
            nc.sync.dma_start(x0_sb[:, 2:4], xT_r[:, 2:4, tok0])
            nc.sync.dma_start(w1_sb[:, 0, 2:4], w1_r[:, 0, 2:4])
            x_tiles[0] = x0_sb
            nc.sync.dma_start(w2_sb[:, 0], w2_r[:, 0])
            for e in range(1, E_LOC):
                nc.sync.dma_start(w1_sb[:, e], w1_r[:, e])
                nc.sync.dma_start(w2_sb[:, e], w2_r[:, e])

            last_t = NTOK // TT - 1
            for t in range(NTOK // TT):
                tok = slice(t * TT, (t + 1) * TT)
                if t not in x_tiles:
                    load_x(t)
                x_sb = x_tiles.pop(t)
                hT_tiles = []
                for e in range(E_LOC):
                    hT_sb = hpool.tile([P, n_ht, TT], BF16, name="hT_sb", tag="h")
                    for ht in range(n_ht):
                        ps1 = ps1p.tile([P, TT], F32, name="ps1", tag="ps1")
                        for dt_i in range(n_dt):
                            nc.tensor.matmul(
                                ps1,
                                w1_sb[:, e, dt_i, ht * P : (ht + 1) * P],
                                x_sb[:, dt_i],
                                start=(dt_i == 0),
                                stop=(dt_i == n_dt - 1),
                            )
                        nc.scalar.activation(hT_sb[:, ht, :], ps1, gelu)
                    hT_tiles.append(hT_sb)
                for e in range(E_LOC):
                    ps2 = ps2p.tile([P, TT], F32, name="ps2", tag="ps2")
                    for ht in range(n_ht):
                        nc.tensor.matmul(
                            ps2,
                            w2_sb[:, e, ht, :],
                            hT_tiles[e][:, ht, :],
                            start=(ht == 0),
                            stop=(ht == n_ht - 1),
                        )
                    o_sb = opool.tile([P, TT], BF16, name="o_sb", tag="o")
                    if t == last_t and e == E_LOC - 1:
                        # Final drain: split the last output in half across
                        # DVE and Scalar so copy+DMA pipeline in parallel.
                        h1, h2 = slice(0, TT // 2), slice(TT // 2, TT)
                        nc.vector.tensor_copy(o_sb[:, h1], ps2[:, h1])
                        nc.sync.dma_start(
                            outT[e, :, t * TT : t * TT + TT // 2], o_sb[:, h1]
                        )
                        nc.scalar.copy(o_sb[:, h2], ps2[:, h2])
                        nc.sync.dma_start(
                            outT[e, :, t * TT + TT // 2 : (t + 1) * TT], o_sb[:, h2]
                        )
                    else:
                        nc.vector.tensor_copy(o_sb, ps2)
                        nc.sync.dma_start(outT[e, :, tok], o_sb)

    nc.finalize()
    return nc


_NC = None


def _get_program():
    global _NC
    if _NC is None:
        _NC = _build_program()
    return _NC


def _prep_in_maps(x, w1, w2):
    """Host-side bf16 cast + transpose; returns per-core input maps."""
    X = np.ascontiguousarray(x.reshape(NTOK, D)).astype(np.float32, copy=False)
    xT = np.ascontiguousarray(X.T.astype(NP_BF16))

    in_maps = []
    for c in range(N_CORES):
        w1c = np.ascontiguousarray(
            w1[c * E_LOC : (c + 1) * E_LOC].astype(NP_BF16)
        )
        w2c = np.ascontiguousarray(
            w2[c * E_LOC : (c + 1) * E_LOC].astype(NP_BF16)
        )
        in_maps.append({"xT": xT, "w1": w1c, "w2": w2c})
    return in_maps


def kernel(x: np.ndarray, w1: np.ndarray, w2: np.ndarray, **_) -> np.ndarray:
    """Full inputs in, full output out; expert-parallel across 8 NeuronCores."""
    nc = _get_program()
    in_maps = _prep_in_maps(x, w1, w2)
    res = run_bass_kernel_spmd(nc, in_maps, list(range(N_CORES)))

    full = np.stack(
        [res.results[c]["outT"].astype(np.float32) for c in range(N_CORES)], axis=0
    )
    full = full.reshape(E, D4, NTOK)              # [e, d4, tok]
    out = full.transpose(2, 1, 0)                 # [tok, d4, e]
    return np.ascontiguousarray(out.reshape(4, 2048, D4, E), dtype=np.float32)


# revision 15
# speedup vs baseline: 1.0134x; 1.0067x over previous
"""Expert-parallel MoE MLP kernel for TRN2 (8 NeuronCores).

Reference computation (all experts, dense routing):
    hidden = einsum("bnd,edh->benh", x, w1); hidden = gelu(hidden)
    out    = einsum("benh,ehd->bnde", hidden, w2)        # [b, n, d4, e]

Sharding: expert-parallel, 2 experts per core (16 experts / 8 cores); x is
replicated. Each core computes, for its experts e:
    hT[e] = gelu(W1[e].T @ X.T)        # [h, tok] layout, h on partitions
    outT[e] = W2[e].T @ hT[e]          # [d4, tok] layout
which keeps the contraction dim on SBUF partitions for both matmuls with no
on-device transposes.

The whole data path is bf16 (PSUM accumulation stays f32): bf16 matmuls run
at the same 1 row/cycle as fp32r (216ns vs 227ns per 512-row matmul measured
- fast weight load hides the weight-load bubble fp32r pays), and bf16 halves
all DMA traffic including the output (upcast to f32 on the host; end-to-end
quantization error ~4e-3, well under the 2e-2 gate). fp8 DoubleRow was
measured at only ~2x fp32r MAC rate on this hardware (not the 4x the cost
model claims), which makes the 3-term hi/lo error-compensation scheme the
2e-2 gate requires a net loss - so bf16 it is. DMA is consolidated into few
dma_starts (each costs ~640ns of serialized sequencer config time) with the
first token tile's critical slices queued ahead of everything else. The
[e, d4, tok] device layout is re-interleaved to [b, n, d4, e] on the host.
"""

import sys

import numpy as np

for _p in ("/opt/trn_rl_repo", "/root/.axon_site/_ro/trn_rl_repo"):
    if _p not in sys.path:
        sys.path.append(_p)

import ml_dtypes

import concourse.bacc as bacc
import concourse.mybir as mybir
import concourse.tile as tile
from concourse.bass_utils import run_bass_kernel_spmd

F32 = mybir.dt.float32
BF16 = mybir.dt.bfloat16
NP_BF16 = ml_dtypes.bfloat16

N_CORES = 8
E = 16                 # total experts
E_LOC = E // N_CORES   # experts per core
D = 512                # model dim (contraction of mm1)
H = 512                # hidden dim (contraction of mm2)
D4 = 128               # output dim per expert
NTOK = 4 * 2048        # tokens
TT = 512               # token tile (matmul moving free dim)
P = 128


def _build_program():
    nc = bacc.Bacc("TRN2", target_bir_lowering=False, debug=False)
    xT = nc.declare_dram_parameter("xT", [D, NTOK], BF16, isOutput=False)
    w1 = nc.declare_dram_parameter("w1", [E_LOC, D, H], BF16, isOutput=False)
    w2 = nc.declare_dram_parameter("w2", [E_LOC, H, D4], BF16, isOutput=False)
    outT = nc.declare_dram_parameter("outT", [E_LOC, D4, NTOK], BF16, isOutput=True)

    gelu = mybir.ActivationFunctionType.Gelu
    n_dt = D // P   # 4 k-tiles of mm1
    n_ht = H // P   # 4 k-tiles of mm2

    with tile.TileContext(nc) as tc:
        with (
            tc.tile_pool(name="wpool", bufs=1) as wpool,
            tc.tile_pool(name="xpool", bufs=4) as xpool,
            tc.tile_pool(name="hpool", bufs=2) as hpool,
            tc.tile_pool(name="opool", bufs=4) as opool,
            tc.tile_pool(name="ps1p", bufs=4, space="PSUM") as ps1p,
            tc.tile_pool(name="ps2p", bufs=3, space="PSUM") as ps2p,
        ):
            # Weights resident in SBUF for the whole kernel, natural layout.
            w1_sb = wpool.tile([P, E_LOC, n_dt, H], BF16, name="w1_sb", tag="w1")
            w1_r = w1.rearrange("e (dt p) h -> p e dt h", p=P)
            w2_sb = wpool.tile([P, E_LOC, n_ht, D4], BF16, name="w2_sb", tag="w2")
            w2_r = w2.rearrange("e (ht p) d -> p e ht d", p=P)
            xT_r = xT.rearrange("(dt p) n -> p dt n", p=P)

            x_tiles = {}

            def load_x(t):
                tok = slice(t * TT, (t + 1) * TT)
                x_sb = xpool.tile([P, n_dt, TT], BF16, name="x_sb", tag="x")
                nc.sync.dma_start(x_sb, xT_r[:, :, tok])
                x_tiles[t] = x_sb

            # Startup: the first matmul needs only x0[dt0] + w1[e0][dt0];
            # those two DMAs go first (each dma_start costs ~640ns of serial
            # sequencer config, so the critical ones must lead the queue).
            tok0 = slice(0, TT)
            x0_sb = xpool.tile([P, n_dt, TT], BF16, name="x_sb", tag="x")
            nc.sync.dma_start(x0_sb[:, 0], xT_r[:, 0, tok0])
            nc.sync.dma_start(w1_sb[:, 0, 0], w1_r[:, 0, 0])
            nc.sync.dma_start(x0_sb[:, 1], xT_r[:, 1, tok0])
            nc.sync.dma_start(w1_sb[:, 0, 1], w1_r[:, 0, 1])
            nc.sync.dma_start(x0_sb[:, 2:4], xT_r[:, 2:4, tok0])
            nc.sync.dma_start(w1_sb[:, 0, 2:4], w1_r[:, 0, 2:4])
            x_tiles[0] = x0_sb
            nc.sync.dma_start(w2_sb[:, 0], w2_r[:, 0])
            for e in range(1, E_LOC):
                nc.sync.dma_start(w1_sb[:, e], w1_r[:, e])
                nc.sync.dma_start(w2_sb[:, e], w2_r[:, e])

            # DVFS pre-ramp: the PE clock steps 0.84->1.2->2.37 GHz over ~3us
            # of continuous execution, normally burned at the head of the real
            # stream. Run a short dummy stream during the initial DMA wait so
            # the ramp overlaps it. Shape mirrors the real stream exactly
            # (4-matmul accumulation groups on alternating PSUM-pool banks,
            # varied operand bits via iota) - a prior attempt with
            # per-instruction groups on one bank + constant data made the
            # governor settle at 2.0 GHz for the whole run.
            warm_sb = wpool.tile([P, TT], BF16, name="warm_sb", tag="warm")
            nc.gpsimd.iota(
                warm_sb,
                pattern=[[1, TT]],
                base=0,
                channel_multiplier=3,
                allow_small_or_imprecise_dtypes=True,
            )
            for _g in range(2):
                wps = ps1p.tile([P, TT], F32, name="ps1", tag="ps1")
                for k in range(n_dt):
                    nc.tensor.matmul(
                        wps,
                        warm_sb[:, k * P : (k + 1) * P],
                        warm_sb,
                        start=(k == 0),
                        stop=(k == n_dt - 1),
                    )

            last_t = NTOK // TT - 1
            for t in range(NTOK // TT):
                tok = slice(t * TT, (t + 1) * TT)
                if t not in x_tiles:
                    load_x(t)
                x_sb = x_tiles.pop(t)
                hT_tiles = []
                for e in range(E_LOC):
                    hT_sb = hpool.tile([P, n_ht, TT], BF16, name="hT_sb", tag="h")
                    for ht in range(n_ht):
                        ps1 = ps1p.tile([P, TT], F32, name="ps1", tag="ps1")
                        for dt_i in range(n_dt):
                            nc.tensor.matmul(
                                ps1,
                                w1_sb[:, e, dt_i, ht * P : (ht + 1) * P],
                                x_sb[:, dt_i],
                                start=(dt_i == 0),
                                stop=(dt_i == n_dt - 1),
                            )
                        nc.scalar.activation(hT_sb[:, ht, :], ps1, gelu)
                    hT_tiles.append(hT_sb)
                for e in range(E_LOC):
                    ps2 = ps2p.tile([P, TT], F32, name="ps2", tag="ps2")
                    for ht in range(n_ht):
                        nc.tensor.matmul(
                            ps2,
                            w2_sb[:, e, ht, :],
                            hT_tiles[e][:, ht, :],
                            start=(ht == 0),
                            stop=(ht == n_ht - 1),
                        )
                    o_sb = opool.tile([P, TT], BF16, name="o_sb", tag="o")
                    nc.vector.tensor_copy(o_sb, ps2)
                    nc.sync.dma_start(outT[e, :, tok], o_sb)

    nc.finalize()
    return nc


_NC = None


def _get_program():
    global _NC
    if _NC is None:
        _NC = _build_program()
    return _NC


def _prep_in_maps(x, w1, w2):
    """Host-side bf16 cast + transpose; returns per-core input maps."""
    X = np.ascontiguousarray(x.reshape(NTOK, D)).astype(np.float32, copy=False)
    xT = np.ascontiguousarray(X.T.astype(NP_BF16))

    in_maps = []
    for c in range(N_CORES):
        w1c = np.ascontiguousarray(
            w1[c * E_LOC : (c + 1) * E_LOC].astype(NP_BF16)
        )
        w2c = np.ascontiguousarray(
            w2[c * E_LOC : (c + 1) * E_LOC].astype(NP_BF16)
        )
        in_maps.append({"xT": xT, "w1": w1c, "w2": w2c})
    return in_maps


def kernel(x: np.ndarray, w1: np.ndarray, w2: np.ndarray, **_) -> np.ndarray:
    """Full inputs in, full output out; expert-parallel across 8 NeuronCores."""
    nc = _get_program()
    in_maps = _prep_in_maps(x, w1, w2)
    res = run_bass_kernel_spmd(nc, in_maps, list(range(N_CORES)))

    full = np.stack(
        [res.results[c]["outT"].astype(np.float32) for c in range(N_CORES)], axis=0
    )
    full = full.reshape(E, D4, NTOK)              # [e, d4, tok]
    out = full.transpose(2, 1, 0)                 # [tok, d4, e]
    return np.ascontiguousarray(out.reshape(4, 2048, D4, E), dtype=np.float32)


# revision 16
# speedup vs baseline: 1.0155x; 1.0021x over previous
"""Expert-parallel MoE MLP kernel for TRN2 (8 NeuronCores).

Reference computation (all experts, dense routing):
    hidden = einsum("bnd,edh->benh", x, w1); hidden = gelu(hidden)
    out    = einsum("benh,ehd->bnde", hidden, w2)        # [b, n, d4, e]

Sharding: expert-parallel, 2 experts per core (16 experts / 8 cores); x is
replicated. Each core computes, for its experts e:
    hT[e] = gelu(W1[e].T @ X.T)        # [h, tok] layout, h on partitions
    outT[e] = W2[e].T @ hT[e]          # [d4, tok] layout
which keeps the contraction dim on SBUF partitions for both matmuls with no
on-device transposes.

The whole data path is bf16 (PSUM accumulation stays f32): bf16 matmuls run
at the same 1 row/cycle as fp32r (216ns vs 227ns per 512-row matmul measured
- fast weight load hides the weight-load bubble fp32r pays), and bf16 halves
all DMA traffic including the output (upcast to f32 on the host; end-to-end
quantization error ~4e-3, well under the 2e-2 gate). fp8 DoubleRow was
measured at only ~2x fp32r MAC rate on this hardware (not the 4x the cost
model claims), which makes the 3-term hi/lo error-compensation scheme the
2e-2 gate requires a net loss - so bf16 it is. DMA is consolidated into few
dma_starts (each costs ~640ns of serialized sequencer config time) with the
first token tile's critical slices queued ahead of everything else. The
[e, d4, tok] device layout is re-interleaved to [b, n, d4, e] on the host.
"""

import sys

import numpy as np

for _p in ("/opt/trn_rl_repo", "/root/.axon_site/_ro/trn_rl_repo"):
    if _p not in sys.path:
        sys.path.append(_p)

import ml_dtypes

import concourse.bacc as bacc
import concourse.mybir as mybir
import concourse.tile as tile
from concourse.bass_utils import run_bass_kernel_spmd

F32 = mybir.dt.float32
BF16 = mybir.dt.bfloat16
NP_BF16 = ml_dtypes.bfloat16

N_CORES = 8
E = 16                 # total experts
E_LOC = E // N_CORES   # experts per core
D = 512                # model dim (contraction of mm1)
H = 512                # hidden dim (contraction of mm2)
D4 = 128               # output dim per expert
NTOK = 4 * 2048        # tokens
TT = 512               # token tile (matmul moving free dim)
P = 128


def _build_program():
    nc = bacc.Bacc("TRN2", target_bir_lowering=False, debug=False)
    xT = nc.declare_dram_parameter("xT", [D, NTOK], BF16, isOutput=False)
    w1 = nc.declare_dram_parameter("w1", [E_LOC, D, H], BF16, isOutput=False)
    w2 = nc.declare_dram_parameter("w2", [E_LOC, H, D4], BF16, isOutput=False)
    outT = nc.declare_dram_parameter("outT", [E_LOC, D4, NTOK], BF16, isOutput=True)

    gelu = mybir.ActivationFunctionType.Gelu
    n_dt = D // P   # 4 k-tiles of mm1
    n_ht = H // P   # 4 k-tiles of mm2

    with tile.TileContext(nc) as tc:
        with (
            tc.tile_pool(name="wpool", bufs=1) as wpool,
            tc.tile_pool(name="xpool", bufs=4) as xpool,
            tc.tile_pool(name="hpool", bufs=2) as hpool,
            tc.tile_pool(name="opool", bufs=4) as opool,
            tc.tile_pool(name="ps1p", bufs=4, space="PSUM") as ps1p,
            tc.tile_pool(name="ps2p", bufs=3, space="PSUM") as ps2p,
        ):
            # Weights resident in SBUF for the whole kernel, natural layout.
            w1_sb = wpool.tile([P, E_LOC, n_dt, H], BF16, name="w1_sb", tag="w1")
            w1_r = w1.rearrange("e (dt p) h -> p e dt h", p=P)
            w2_sb = wpool.tile([P, E_LOC, n_ht, D4], BF16, name="w2_sb", tag="w2")
            w2_r = w2.rearrange("e (ht p) d -> p e ht d", p=P)
            xT_r = xT.rearrange("(dt p) n -> p dt n", p=P)

            x_tiles = {}

            def load_x(t):
                tok = slice(t * TT, (t + 1) * TT)
                x_sb = xpool.tile([P, n_dt, TT], BF16, name="x_sb", tag="x")
                nc.sync.dma_start(x_sb, xT_r[:, :, tok])
                x_tiles[t] = x_sb

            # Startup: the first matmul needs only x0[dt0] + w1[e0][dt0];
            # those two DMAs go first (each dma_start costs ~640ns of serial
            # sequencer config, so the critical ones must lead the queue).
            tok0 = slice(0, TT)
            x0_sb = xpool.tile([P, n_dt, TT], BF16, name="x_sb", tag="x")
            nc.sync.dma_start(x0_sb[:, 0], xT_r[:, 0, tok0])
            nc.sync.dma_start(w1_sb[:, 0, 0], w1_r[:, 0, 0])
            nc.sync.dma_start(x0_sb[:, 1], xT_r[:, 1, tok0])
            nc.sync.dma_start(w1_sb[:, 0, 1], w1_r[:, 0, 1])
            nc.sync.dma_start(x0_sb[:, 2:4], xT_r[:, 2:4, tok0])
            nc.sync.dma_start(w1_sb[:, 0, 2:4], w1_r[:, 0, 2:4])
            x_tiles[0] = x0_sb
            nc.sync.dma_start(w2_sb[:, 0], w2_r[:, 0])
            for e in range(1, E_LOC):
                nc.sync.dma_start(w1_sb[:, e], w1_r[:, e])
                nc.sync.dma_start(w2_sb[:, e], w2_r[:, e])

            # DVFS pre-ramp: the PE clock steps 0.84->1.2->2.37 GHz over ~3us
            # of continuous execution, normally burned at the head of the real
            # stream. Run a short dummy stream during the initial DMA wait so
            # the ramp overlaps it. Shape mirrors the real stream exactly
            # (4-matmul accumulation groups on alternating PSUM-pool banks,
            # varied operand bits via iota) - a prior attempt with
            # per-instruction groups on one bank + constant data made the
            # governor settle at 2.0 GHz for the whole run.
            warm_sb = wpool.tile([P, TT], BF16, name="warm_sb", tag="warm")
            nc.gpsimd.iota(
                warm_sb,
                pattern=[[1, TT]],
                base=0,
                channel_multiplier=3,
                allow_small_or_imprecise_dtypes=True,
            )
            # One long gapless accumulation group: any gap in the stream
            # resets the ~3us ramp timer, so the dummies must cover the whole
            # DMA wait in one unbroken run, ending only once tile 0's data
            # has fully landed (the real stream then starts gapless at full
            # clock instead of hitting DMA-wait gaps mid-ramp).
            n_warm = 9
            wps = ps1p.tile([P, TT], F32, name="ps1", tag="ps1")
            for k in range(n_warm):
                nc.tensor.matmul(
                    wps,
                    warm_sb[:, (k % n_dt) * P : (k % n_dt + 1) * P],
                    warm_sb,
                    start=(k == 0),
                    stop=(k == n_warm - 1),
                )

            last_t = NTOK // TT - 1
            for t in range(NTOK // TT):
                tok = slice(t * TT, (t + 1) * TT)
                if t not in x_tiles:
                    load_x(t)
                x_sb = x_tiles.pop(t)
                hT_tiles = []
                for e in range(E_LOC):
                    hT_sb = hpool.tile([P, n_ht, TT], BF16, name="hT_sb", tag="h")
                    for ht in range(n_ht):
                        ps1 = ps1p.tile([P, TT], F32, name="ps1", tag="ps1")
                        for dt_i in range(n_dt):
                            nc.tensor.matmul(
                                ps1,
                                w1_sb[:, e, dt_i, ht * P : (ht + 1) * P],
                                x_sb[:, dt_i],
                                start=(dt_i == 0),
                                stop=(dt_i == n_dt - 1),
                            )
                        nc.scalar.activation(hT_sb[:, ht, :], ps1, gelu)
                    hT_tiles.append(hT_sb)
                for e in range(E_LOC):
                    ps2 = ps2p.tile([P, TT], F32, name="ps2", tag="ps2")
                    for ht in range(n_ht):
                        nc.tensor.matmul(
                            ps2,
                            w2_sb[:, e, ht, :],
                            hT_tiles[e][:, ht, :],
                            start=(ht == 0),
                            stop=(ht == n_ht - 1),
                        )
                    o_sb = opool.tile([P, TT], BF16, name="o_sb", tag="o")
                    nc.vector.tensor_copy(o_sb, ps2)
                    nc.sync.dma_start(outT[e, :, tok], o_sb)

    nc.finalize()
    return nc


_NC = None


def _get_program():
    global _NC
    if _NC is None:
        _NC = _build_program()
    return _NC


def _prep_in_maps(x, w1, w2):
    """Host-side bf16 cast + transpose; returns per-core input maps."""
    X = np.ascontiguousarray(x.reshape(NTOK, D)).astype(np.float32, copy=False)
    xT = np.ascontiguousarray(X.T.astype(NP_BF16))

    in_maps = []
    for c in range(N_CORES):
        w1c = np.ascontiguousarray(
            w1[c * E_LOC : (c + 1) * E_LOC].astype(NP_BF16)
        )
        w2c = np.ascontiguousarray(
            w2[c * E_LOC : (c + 1) * E_LOC].astype(NP_BF16)
        )
        in_maps.append({"xT": xT, "w1": w1c, "w2": w2c})
    return in_maps


def kernel(x: np.ndarray, w1: np.ndarray, w2: np.ndarray, **_) -> np.ndarray:
    """Full inputs in, full output out; expert-parallel across 8 NeuronCores."""
    nc = _get_program()
    in_maps = _prep_in_maps(x, w1, w2)
    res = run_bass_kernel_spmd(nc, in_maps, list(range(N_CORES)))

    full = np.stack(
        [res.results[c]["outT"].astype(np.float32) for c in range(N_CORES)], axis=0
    )
    full = full.reshape(E, D4, NTOK)              # [e, d4, tok]
    out = full.transpose(2, 1, 0)                 # [tok, d4, e]
    return np.ascontiguousarray(out.reshape(4, 2048, D4, E), dtype=np.float32)
